# revision 3
# baseline (speedup 1.0000x reference)
"""Causal self-attention Bass/Tile kernel for Trainium2, SPMD over 8 NeuronCores.

Problem: B=4, T=2048, C=768, NH=12 heads, D=64. y = softmax(mask(qk^T/sqrt(D))) v,
with qkv = x@W_attn + b_attn and out = y@W_proj + b_proj.

Sharding: core c handles batch b = c//2 and heads [hs, hs+6) where hs = (c%2)*6.
Each core computes a partial output part_c = y_c @ W_proj[rows of its heads]; the
host sums the two partials of each batch pair (fp32) and adds b_proj.

v3 design (vs the fp32r v1 baseline):
  - bf16 operands everywhere on the matmul paths (q/k/v/P/y/weights); fp32 PSUM
    accumulation. Halves HBM traffic and SBUF footprint; enables FWL weight loads.
  - t-chunk-major emission: A(qkv proj), B(attention) and C(out proj) are emitted
    interleaved (A0 A1 B0 A2 B1 C0 A3 B2 C1 B3 C2 C3) so the tensor engine never
    drains between phases and the ACT-paced B segments overlap A/C matmul work.
  - software-pipelined B inner loop: scores matmuls for s-block group g+1 are
    emitted before the AV matmuls of group g, hiding the exp (ACT) latency.
  - exp on [128,1024] blocks (two s-blocks per activation) to amortize the
    ~352-cycle ACT instruction overhead; causal masking via gpsimd affine_select
    on the bf16 P tile (diag blocks only); softmax denominator via an extra
    ones-column in the packed v operand (row 64 of the yT_aug accumulator).
  - bf16 partial outputs; host accumulates in fp32.
"""

import contextlib
import os
import sys

for _p in ("/opt/trn_rl_repo", "/root/.axon_site/_ro/trn_rl_repo"):
    if os.path.isdir(_p) and _p not in sys.path:
        sys.path.insert(0, _p)
        break

import numpy as np
import ml_dtypes

import concourse.bass as bass  # noqa: F401
import concourse.mybir as mybir
import concourse.tile as tile
from concourse import bacc
from concourse.bass_utils import run_bass_kernel_spmd

FP32 = mybir.dt.float32
FP32R = mybir.dt.float32r
BF16 = mybir.dt.bfloat16

B, T, C = 4, 2048, 768
NH, D = 12, 64
NCORES = 8
NKC = C // 128          # 6 contraction chunks
NTB = T // 128          # 16 s blocks
TCW = 512
NTC = T // TCW          # 4 t chunks
HPC = 6                 # heads per core
VW = HPC * 65           # 390: v block row width (6 heads x (64 + ones col))
WQKV = 3 * HPC * D      # 1152

_BUILT = None


def _build_nc(loops=1, phases="ABC"):
    nc = bacc.Bacc("TRN2", target_bir_lowering=False, debug=False, num_devices=NCORES)

    xT = nc.dram_tensor("xT", [C, T], BF16, kind="ExternalInput")
    wqkv = nc.dram_tensor("wqkv", [C, WQKV], BF16, kind="ExternalInput")
    bqk = nc.dram_tensor("bqk", [128, 6], FP32, kind="ExternalInput")
    bvb = nc.dram_tensor("bvb", [128, 384], FP32, kind="ExternalInput")
    wproj = nc.dram_tensor("wproj", [384, C], BF16, kind="ExternalInput")
    part = nc.dram_tensor("part", [T, C], BF16, kind="ExternalOutput")

    with tile.TileContext(nc) as tc:
        def loop_cm(n):
            return tc.For_i(0, n, 1) if n > 1 else contextlib.nullcontext()

        with tc.sbuf_pool(name="pers", bufs=1) as pers, \
             tc.sbuf_pool(name="work", bufs=1) as work, \
             tc.psum_pool(name="ps", bufs=1) as ps:
            xt = pers.tile([128, NKC * T], BF16)
            wa = pers.tile([128, NKC * WQKV], BF16)
            qT = pers.tile([128, 3 * T], BF16)
            kT = pers.tile([128, 3 * T], BF16)
            vaug = pers.tile([128, NTB * VW], BF16)
            yT = pers.tile([128, 3 * T], BF16)
            wp = pers.tile([128, 3 * C], BF16)
            bqk_sb = pers.tile([128, 6], FP32)
            bvb_sb = pers.tile([128, 384], FP32)

            nc.sync.dma_start(
                out=wp.rearrange("p (k c) -> p k c", c=C),
                in_=wproj.rearrange("(k p) c -> p k c", p=128),
            )
            nc.sync.dma_start(out=bqk_sb, in_=bqk[:, :])
            nc.sync.dma_start(out=bvb_sb, in_=bvb[:, :])
            ones_cols = vaug.rearrange("p (g c) -> p g c", c=65)[:, :, 64:65]
            nc.vector.memset(ones_cols, 1.0)

            def emit_A(tcw):
                # q,k projections for this t-chunk (3 head pairs x {q,k})
                for pp in range(3):
                    for qk in range(2):
                        acc = ps.tile([128, TCW], FP32, tag="mm", bufs=2)
                        for kc in range(NKC):
                            co = kc * WQKV + qk * 384 + pp * 128
                            nc.tensor.matmul(
                                acc,
                                wa[:, co:co + 128],
                                xt[:, kc * T + tcw * TCW: kc * T + (tcw + 1) * TCW],
                                start=(kc == 0),
                                stop=(kc == NKC - 1),
                            )
                        dst = qT if qk == 0 else kT
                        nc.vector.tensor_scalar_add(
                            dst[:, pp * T + tcw * TCW: pp * T + (tcw + 1) * TCW],
                            acc,
                            bqk_sb[:, 3 * qk + pp: 3 * qk + pp + 1],
                        )
                # v projection for this t-chunk's 4 s-blocks
                for tb in range(4 * tcw, 4 * tcw + 4):
                    accv = ps.tile([128, TCW], FP32, tag="mm", bufs=2)
                    for kc in range(NKC):
                        nc.tensor.matmul(
                            accv[:, 0:384],
                            xt[:, kc * T + tb * 128: kc * T + (tb + 1) * 128],
                            wa[:, kc * WQKV + 768: kc * WQKV + WQKV],
                            start=(kc == 0),
                            stop=(kc == NKC - 1),
                        )
                    vdst = vaug[:, tb * VW:(tb + 1) * VW]
                    nc.vector.tensor_tensor(
                        out=vdst.rearrange("p (h c) -> p h c", c=65)[:, :, 0:64],
                        in0=accv[:, 0:384].rearrange("p (h c) -> p h c", c=64),
                        in1=bvb_sb.rearrange("p (h c) -> p h c", c=64),
                        op=mybir.AluOpType.add,
                    )

            def emit_B(tcw):
                nsb = 4 * tcw + 4
                for pp in range(3):
                    yps = [ps.tile([65, TCW], FP32, tag="yt", bufs=2,
                                   name=f"yps{e}")
                           for e in range(2)]

                    def scores_group(g):
                        """4 score MMs + 2 exps (+ affine masks); returns P per e."""
                        out = []
                        for e in range(2):
                            pt = ps.tile([128, 2 * TCW], FP32, tag="pt", bufs=2)
                            for j in range(2):
                                sb = 2 * g + j
                                nc.tensor.matmul(
                                    pt[:, j * TCW:(j + 1) * TCW],
                                    kT[e * 64:(e + 1) * 64,
                                       pp * T + sb * 128: pp * T + (sb + 1) * 128],
                                    qT[e * 64:(e + 1) * 64,
                                       pp * T + tcw * TCW: pp * T + (tcw + 1) * TCW],
                                    start=True,
                                    stop=True,
                                )
                            P = work.tile([128, 2 * TCW], BF16, tag="p", bufs=4)
                            nc.scalar.activation(
                                P, pt, mybir.ActivationFunctionType.Exp, scale=0.125,
                            )
                            for j in range(2):
                                sb = 2 * g + j
                                if sb >= 4 * tcw:  # diagonal-band block
                                    psl = P[:, j * TCW:(j + 1) * TCW]
                                    nc.gpsimd.affine_select(
                                        out=psl, in_=psl,
                                        compare_op=mybir.AluOpType.is_ge,
                                        fill=0.0,
                                        base=tcw * TCW - sb * 128,
                                        channel_multiplier=-1,
                                        pattern=[[1, TCW]],
                                    )
                            out.append(P)
                        return out

                    def av_group(g, Ps):
                        for e in range(2):
                            h = 2 * pp + e
                            for j in range(2):
                                sb = 2 * g + j
                                nc.tensor.matmul(
                                    yps[e],
                                    vaug[:, sb * VW + h * 65: sb * VW + (h + 1) * 65],
                                    Ps[e][:, j * TCW:(j + 1) * TCW],
                                    start=(sb == 0),
                                    stop=(sb == nsb - 1),
                                )

                    prev = None
                    for g in range(nsb // 2):
                        cur = (g, scores_group(g))
                        if prev is not None:
                            av_group(*prev)
                        prev = cur
                    av_group(*prev)

                    for e in range(2):
                        rt = work.tile([1, TCW], FP32R, tag="rt", bufs=2)
                        with nc.allow_low_precision(reason="fp32r softmax denom"):
                            nc.vector.reciprocal(rt, yps[e][64:65, :])
                        rbc = work.tile([64, TCW], FP32R, tag="rbc", bufs=2)
                        nc.gpsimd.partition_broadcast(rbc, rt)
                        with nc.allow_low_precision(reason="bf16 attn out"):
                            nc.vector.tensor_tensor(
                                out=yT[e * 64:(e + 1) * 64,
                                       pp * T + tcw * TCW: pp * T + (tcw + 1) * TCW],
                                in0=yps[e][0:64, :],
                                in1=rbc,
                                op=mybir.AluOpType.mult,
                            )

            def emit_C(tcw):
                for tb in range(4 * tcw, 4 * tcw + 4):
                    osb = work.tile([128, C], BF16, tag="osb", bufs=2)
                    for ncw in range(2):
                        acc = ps.tile([128, TCW], FP32, tag="mm", bufs=2)
                        for cc in range(3):
                            nc.tensor.matmul(
                                acc[:, 0:384],
                                yT[:, cc * T + tb * 128: cc * T + (tb + 1) * 128],
                                wp[:, cc * C + ncw * 384: cc * C + (ncw + 1) * 384],
                                start=(cc == 0),
                                stop=(cc == 2),
                            )
                        nc.vector.tensor_copy(
                            osb[:, ncw * 384:(ncw + 1) * 384], acc[:, 0:384])
                    nc.sync.dma_start(
                        out=part[tb * 128:(tb + 1) * 128, :], in_=osb
                    )

            with loop_cm(loops):
                # input DMAs, weights first then x t-chunk-major
                for kc in range(NKC):
                    nc.sync.dma_start(
                        out=wa[:, kc * WQKV:(kc + 1) * WQKV],
                        in_=wqkv[kc * 128:(kc + 1) * 128, :],
                    )
                for tcw in range(NTC):
                    for kc in range(NKC):
                        nc.sync.dma_start(
                            out=xt[:, kc * T + tcw * TCW: kc * T + (tcw + 1) * TCW],
                            in_=xT[kc * 128:(kc + 1) * 128,
                                   tcw * TCW:(tcw + 1) * TCW],
                        )
                if "B" in phases:
                    emit_A(0)
                    emit_A(1)
                    for tcw in range(NTC):
                        emit_B(tcw)
                        if tcw + 2 < NTC:
                            emit_A(tcw + 2)
                        if "C" in phases and tcw >= 1:
                            emit_C(tcw - 1)
                    if "C" in phases:
                        emit_C(NTC - 1)
                else:  # A only (timing variant)
                    for tcw in range(NTC):
                        emit_A(tcw)
                if "C" not in phases or "B" not in phases:
                    z = work.tile([128, 512], BF16, tag="osb0", bufs=1)
                    nc.vector.memset(z, 0.0)
                    nc.sync.dma_start(out=part[0:128, 0:512], in_=z)

    nc.compile()
    return nc


def _get_nc():
    global _BUILT
    if _BUILT is None:
        _BUILT = _build_nc()
    return _BUILT


def build_in_maps(x, W_attn, b_attn, W_proj):
    bf = ml_dtypes.bfloat16
    in_maps = []
    for c in range(NCORES):
        b = c // 2
        hs = (c % 2) * HPC
        q0, k0, v0 = hs * D, C + hs * D, 2 * C + hs * D
        w = HPC * D  # 384
        xT_b = np.ascontiguousarray(x[b].T).astype(bf)
        wqkv_c = np.ascontiguousarray(
            np.concatenate(
                [W_attn[:, q0:q0 + w], W_attn[:, k0:k0 + w], W_attn[:, v0:v0 + w]],
                axis=1,
            )
        ).astype(bf)
        bqk_c = np.stack(
            [b_attn[q0 + pp * 128: q0 + (pp + 1) * 128] for pp in range(3)]
            + [b_attn[k0 + pp * 128: k0 + (pp + 1) * 128] for pp in range(3)],
            axis=1,
        ).astype(np.float32)
        bvb_c = np.ascontiguousarray(
            np.broadcast_to(b_attn[v0:v0 + w][None, :], (128, w))
        ).astype(np.float32)
        wproj_c = np.ascontiguousarray(W_proj[hs * D: hs * D + w, :]).astype(bf)
        in_maps.append(
            {
                "xT": xT_b,
                "wqkv": wqkv_c,
                "bqk": bqk_c,
                "bvb": bvb_c,
                "wproj": wproj_c,
            }
        )
    return in_maps


def kernel(x, W_attn, b_attn, W_proj, b_proj):
    x = np.asarray(x, dtype=np.float32)
    W_attn = np.asarray(W_attn, dtype=np.float32)
    b_attn = np.asarray(b_attn, dtype=np.float32)
    W_proj = np.asarray(W_proj, dtype=np.float32)
    b_proj = np.asarray(b_proj, dtype=np.float32)

    nc = _get_nc()
    in_maps = build_in_maps(x, W_attn, b_attn, W_proj)
    res = run_bass_kernel_spmd(nc, in_maps, core_ids=list(range(NCORES)))
    out = np.empty((B, T, C), dtype=np.float32)
    for b in range(B):
        out[b] = (
            res.results[2 * b]["part"].astype(np.float32)
            + res.results[2 * b + 1]["part"].astype(np.float32)
            + b_proj[None, :]
        )
    return out


# revision 4
# speedup vs baseline: 4752.9205x; 4752.9205x over previous
"""Causal self-attention Bass/Tile kernel for Trainium2, SPMD over 8 NeuronCores.

Problem: B=4, T=2048, C=768, NH=12 heads, D=64. y = softmax(mask(qk^T/sqrt(D))) v,
with qkv = x@W_attn + b_attn and out = y@W_proj + b_proj.

Sharding: core c handles batch b = c//2 and heads [hs, hs+6) where hs = (c%2)*6.
Each core computes a partial output part_c = y_c @ W_proj[rows of its heads]; the
host sums the two partials of each batch pair (fp32) and adds b_proj.

v3 design (vs the fp32r v1 baseline):
  - bf16 operands everywhere on the matmul paths (q/k/v/P/y/weights); fp32 PSUM
    accumulation. Halves HBM traffic and SBUF footprint; enables FWL weight loads.
  - t-chunk-major emission: A(qkv proj), B(attention) and C(out proj) are emitted
    interleaved (A0 A1 B0 A2 B1 C0 A3 B2 C1 B3 C2 C3) so the tensor engine never
    drains between phases and the ACT-paced B segments overlap A/C matmul work.
  - software-pipelined B inner loop: scores matmuls for s-block group g+1 are
    emitted before the AV matmuls of group g, hiding the exp (ACT) latency.
  - exp on [128,1024] blocks (two s-blocks per activation) to amortize the
    ~352-cycle ACT instruction overhead; causal masking via gpsimd affine_select
    on the bf16 P tile (diag blocks only); softmax denominator via an extra
    ones-column in the packed v operand (row 64 of the yT_aug accumulator).
  - bf16 partial outputs; host accumulates in fp32.
"""

import contextlib
import os
import sys

for _p in ("/opt/trn_rl_repo", "/root/.axon_site/_ro/trn_rl_repo"):
    if os.path.isdir(_p) and _p not in sys.path:
        sys.path.insert(0, _p)
        break

import numpy as np
import ml_dtypes

import concourse.bass as bass  # noqa: F401
import concourse.mybir as mybir
import concourse.tile as tile
from concourse import bacc
from concourse.bass_utils import run_bass_kernel_spmd

FP32 = mybir.dt.float32
FP32R = mybir.dt.float32r
BF16 = mybir.dt.bfloat16

B, T, C = 4, 2048, 768
NH, D = 12, 64
NCORES = 8
NKC = C // 128          # 6 contraction chunks
NTB = T // 128          # 16 s blocks
TCW = 512
NTC = T // TCW          # 4 t chunks
HPC = 6                 # heads per core
VW = HPC * 65           # 390: v block row width (6 heads x (64 + ones col))
WQKV = 3 * HPC * D      # 1152

_BUILT = None


def _build_nc(loops=1, phases="ABC"):
    nc = bacc.Bacc("TRN2", target_bir_lowering=False, debug=False, num_devices=NCORES)

    xT = nc.dram_tensor("xT", [C, T], BF16, kind="ExternalInput")
    wqkv = nc.dram_tensor("wqkv", [C, WQKV], BF16, kind="ExternalInput")
    bqk = nc.dram_tensor("bqk", [128, 6], FP32, kind="ExternalInput")
    bvb = nc.dram_tensor("bvb", [128, 384], FP32, kind="ExternalInput")
    wproj = nc.dram_tensor("wproj", [384, C], BF16, kind="ExternalInput")
    part = nc.dram_tensor("part", [T, C], BF16, kind="ExternalOutput")

    with tile.TileContext(nc) as tc:
        def loop_cm(n):
            return tc.For_i(0, n, 1) if n > 1 else contextlib.nullcontext()

        with tc.sbuf_pool(name="pers", bufs=1) as pers, \
             tc.sbuf_pool(name="work", bufs=1) as work, \
             tc.psum_pool(name="ps", bufs=1) as ps:
            xt = pers.tile([128, NKC * T], BF16)
            wa = pers.tile([128, NKC * WQKV], BF16)
            qT = pers.tile([128, 3 * T], BF16)
            kT = pers.tile([128, 3 * T], BF16)
            vaug = pers.tile([128, NTB * VW], BF16)
            yT = pers.tile([128, 3 * T], BF16)
            wp = pers.tile([128, 3 * C], BF16)
            bqk_sb = pers.tile([128, 6], FP32)
            bvb_sb = pers.tile([128, 384], FP32)

            nc.sync.dma_start(
                out=wp.rearrange("p (k c) -> p k c", c=C),
                in_=wproj.rearrange("(k p) c -> p k c", p=128),
            )
            nc.sync.dma_start(out=bqk_sb, in_=bqk[:, :])
            nc.sync.dma_start(out=bvb_sb, in_=bvb[:, :])
            ones_cols = vaug.rearrange("p (g c) -> p g c", c=65)[:, :, 64:65]
            nc.vector.memset(ones_cols, 1.0)

            def a_qk_chain(tcw, pp, qk):
                acc = ps.tile([128, TCW], FP32, tag="mm", bufs=2, name="accqk")
                for kc in range(NKC):
                    co = kc * WQKV + qk * 384 + pp * 128
                    nc.tensor.matmul(
                        acc,
                        wa[:, co:co + 128],
                        xt[:, kc * T + tcw * TCW: kc * T + (tcw + 1) * TCW],
                        start=(kc == 0),
                        stop=(kc == NKC - 1),
                    )
                dst = qT if qk == 0 else kT
                nc.vector.tensor_scalar_add(
                    dst[:, pp * T + tcw * TCW: pp * T + (tcw + 1) * TCW],
                    acc,
                    bqk_sb[:, 3 * qk + pp: 3 * qk + pp + 1],
                )

            def a_v_chain(tb):
                accv = ps.tile([128, TCW], FP32, tag="mm", bufs=2, name="accv")
                for kc in range(NKC):
                    nc.tensor.matmul(
                        accv[:, 0:384],
                        xt[:, kc * T + tb * 128: kc * T + (tb + 1) * 128],
                        wa[:, kc * WQKV + 768: kc * WQKV + WQKV],
                        start=(kc == 0),
                        stop=(kc == NKC - 1),
                    )
                vdst = vaug[:, tb * VW:(tb + 1) * VW]
                nc.vector.tensor_tensor(
                    out=vdst.rearrange("p (h c) -> p h c", c=65)[:, :, 0:64],
                    in0=accv[:, 0:384].rearrange("p (h c) -> p h c", c=64),
                    in1=bvb_sb.rearrange("p (h c) -> p h c", c=64),
                    op=mybir.AluOpType.add,
                )

            def a_units(tcw):
                us = []
                for pp in range(3):
                    for qk in range(2):
                        us.append(lambda pp=pp, qk=qk: a_qk_chain(tcw, pp, qk))
                for tb in range(4 * tcw, 4 * tcw + 4):
                    us.append(lambda tb=tb: a_v_chain(tb))
                return us

            def c_tb(tb):
                osb = work.tile([128, C], BF16, tag="osb", bufs=2, name="osb")
                for ncw in range(2):
                    acc = ps.tile([128, TCW], FP32, tag="mm", bufs=2, name="accc")
                    for cc in range(3):
                        nc.tensor.matmul(
                            acc[:, 0:384],
                            yT[:, cc * T + tb * 128: cc * T + (tb + 1) * 128],
                            wp[:, cc * C + ncw * 384: cc * C + (ncw + 1) * 384],
                            start=(cc == 0),
                            stop=(cc == 2),
                        )
                    nc.vector.tensor_copy(
                        osb[:, ncw * 384:(ncw + 1) * 384], acc[:, 0:384])
                nc.sync.dma_start(out=part[tb * 128:(tb + 1) * 128, :], in_=osb)

            def c_units(tcw):
                return [lambda tb=tb: c_tb(tb)
                        for tb in range(4 * tcw, 4 * tcw + 4)]

            def emit_B(tcw, fillers=()):
                nsb = 4 * tcw + 4
                for pp in range(3):
                    yps = [ps.tile([65, TCW], FP32, tag="yt", bufs=2,
                                   name=f"yps{e}")
                           for e in range(2)]

                    def scores_group(g):
                        """4 score MMs + 2 exps (+ affine masks); returns P per e."""
                        out = []
                        for e in range(2):
                            pt = ps.tile([128, 2 * TCW], FP32, tag="pt", bufs=2)
                            for j in range(2):
                                sb = 2 * g + j
                                nc.tensor.matmul(
                                    pt[:, j * TCW:(j + 1) * TCW],
                                    kT[e * 64:(e + 1) * 64,
                                       pp * T + sb * 128: pp * T + (sb + 1) * 128],
                                    qT[e * 64:(e + 1) * 64,
                                       pp * T + tcw * TCW: pp * T + (tcw + 1) * TCW],
                                    start=True,
                                    stop=True,
                                )
                            P = work.tile([128, 2 * TCW], BF16, tag="p", bufs=4)
                            nc.scalar.activation(
                                P, pt, mybir.ActivationFunctionType.Exp, scale=0.125,
                            )
                            for j in range(2):
                                sb = 2 * g + j
                                if sb >= 4 * tcw:  # diagonal-band block
                                    psl = P[:, j * TCW:(j + 1) * TCW]
                                    nc.gpsimd.affine_select(
                                        out=psl, in_=psl,
                                        compare_op=mybir.AluOpType.is_ge,
                                        fill=0.0,
                                        base=tcw * TCW - sb * 128,
                                        channel_multiplier=-1,
                                        pattern=[[1, TCW]],
                                    )
                            out.append(P)
                        return out

                    def av_group(g, Ps):
                        for e in range(2):
                            h = 2 * pp + e
                            for j in range(2):
                                sb = 2 * g + j
                                nc.tensor.matmul(
                                    yps[e],
                                    vaug[:, sb * VW + h * 65: sb * VW + (h + 1) * 65],
                                    Ps[e][:, j * TCW:(j + 1) * TCW],
                                    start=(sb == 0),
                                    stop=(sb == nsb - 1),
                                )

                    prev = None
                    for g in range(nsb // 2):
                        cur = (g, scores_group(g))
                        if prev is not None:
                            av_group(*prev)
                        prev = cur
                    av_group(*prev)

                    for e in range(2):
                        rt = work.tile([1, TCW], FP32R, tag="rt", bufs=2)
                        with nc.allow_low_precision(reason="fp32r softmax denom"):
                            nc.vector.reciprocal(rt, yps[e][64:65, :])
                        rbc = work.tile([64, TCW], FP32R, tag="rbc", bufs=2)
                        nc.gpsimd.partition_broadcast(rbc, rt)
                        with nc.allow_low_precision(reason="bf16 attn out"):
                            nc.vector.tensor_tensor(
                                out=yT[e * 64:(e + 1) * 64,
                                       pp * T + tcw * TCW: pp * T + (tcw + 1) * TCW],
                                in0=yps[e][0:64, :],
                                in1=rbc,
                                op=mybir.AluOpType.mult,
                            )

            def emit_C(tcw):
                for tb in range(4 * tcw, 4 * tcw + 4):
                    osb = work.tile([128, C], BF16, tag="osb", bufs=2)
                    for ncw in range(2):
                        acc = ps.tile([128, TCW], FP32, tag="mm", bufs=2)
                        for cc in range(3):
                            nc.tensor.matmul(
                                acc[:, 0:384],
                                yT[:, cc * T + tb * 128: cc * T + (tb + 1) * 128],
                                wp[:, cc * C + ncw * 384: cc * C + (ncw + 1) * 384],
                                start=(cc == 0),
                                stop=(cc == 2),
                            )
                        nc.vector.tensor_copy(
                            osb[:, ncw * 384:(ncw + 1) * 384], acc[:, 0:384])
                    nc.sync.dma_start(
                        out=part[tb * 128:(tb + 1) * 128, :], in_=osb
                    )

            with loop_cm(loops):
                # input DMAs, weights first then x t-chunk-major
                for kc in range(NKC):
                    nc.sync.dma_start(
                        out=wa[:, kc * WQKV:(kc + 1) * WQKV],
                        in_=wqkv[kc * 128:(kc + 1) * 128, :],
                    )
                for tcw in range(NTC):
                    for kc in range(NKC):
                        nc.sync.dma_start(
                            out=xt[:, kc * T + tcw * TCW: kc * T + (tcw + 1) * TCW],
                            in_=xT[kc * 128:(kc + 1) * 128,
                                   tcw * TCW:(tcw + 1) * TCW],
                        )
                if "B" in phases:
                    emit_A(0)
                    emit_A(1)
                    for tcw in range(NTC):
                        emit_B(tcw)
                        if tcw + 2 < NTC:
                            emit_A(tcw + 2)
                        if "C" in phases and tcw >= 1:
                            emit_C(tcw - 1)
                    if "C" in phases:
                        emit_C(NTC - 1)
                else:  # A only (timing variant)
                    for tcw in range(NTC):
                        emit_A(tcw)
                if "C" not in phases or "B" not in phases:
                    z = work.tile([128, 512], BF16, tag="osb0", bufs=1)
                    nc.vector.memset(z, 0.0)
                    nc.sync.dma_start(out=part[0:128, 0:512], in_=z)

    nc.compile()
    return nc


def _get_nc():
    global _BUILT
    if _BUILT is None:
        _BUILT = _build_nc()
    return _BUILT


def build_in_maps(x, W_attn, b_attn, W_proj):
    bf = ml_dtypes.bfloat16
    in_maps = []
    for c in range(NCORES):
        b = c // 2
        hs = (c % 2) * HPC
        q0, k0, v0 = hs * D, C + hs * D, 2 * C + hs * D
        w = HPC * D  # 384
        xT_b = np.ascontiguousarray(x[b].T).astype(bf)
        wqkv_c = np.ascontiguousarray(
            np.concatenate(
                [W_attn[:, q0:q0 + w], W_attn[:, k0:k0 + w], W_attn[:, v0:v0 + w]],
                axis=1,
            )
        ).astype(bf)
        bqk_c = np.stack(
            [b_attn[q0 + pp * 128: q0 + (pp + 1) * 128] for pp in range(3)]
            + [b_attn[k0 + pp * 128: k0 + (pp + 1) * 128] for pp in range(3)],
            axis=1,
        ).astype(np.float32)
        bvb_c = np.ascontiguousarray(
            np.broadcast_to(b_attn[v0:v0 + w][None, :], (128, w))
        ).astype(np.float32)
        wproj_c = np.ascontiguousarray(W_proj[hs * D: hs * D + w, :]).astype(bf)
        in_maps.append(
            {
                "xT": xT_b,
                "wqkv": wqkv_c,
                "bqk": bqk_c,
                "bvb": bvb_c,
                "wproj": wproj_c,
            }
        )
    return in_maps


def kernel(x, W_attn, b_attn, W_proj, b_proj):
    x = np.asarray(x, dtype=np.float32)
    W_attn = np.asarray(W_attn, dtype=np.float32)
    b_attn = np.asarray(b_attn, dtype=np.float32)
    W_proj = np.asarray(W_proj, dtype=np.float32)
    b_proj = np.asarray(b_proj, dtype=np.float32)

    nc = _get_nc()
    in_maps = build_in_maps(x, W_attn, b_attn, W_proj)
    res = run_bass_kernel_spmd(nc, in_maps, core_ids=list(range(NCORES)))
    out = np.empty((B, T, C), dtype=np.float32)
    for b in range(B):
        out[b] = (
            res.results[2 * b]["part"].astype(np.float32)
            + res.results[2 * b + 1]["part"].astype(np.float32)
            + b_proj[None, :]
        )
    return out


# revision 14
# speedup vs baseline: 4812.9598x; 1.0126x over previous
"""Causal self-attention Bass/Tile kernel for Trainium2, SPMD over 8 NeuronCores.

Problem: B=4, T=2048, C=768, NH=12 heads, D=64. y = softmax(mask(qk^T/sqrt(D))) v,
with qkv = x@W_attn + b_attn and out = y@W_proj + b_proj.

Sharding: core c handles batch b = c//2 and heads [hs, hs+6) where hs = (c%2)*6.
Each core computes a partial output part_c = y_c @ W_proj[rows of its heads]; the
host sums the two partials of each batch pair (fp32) and adds b_proj.

v3 design (vs the fp32r v1 baseline):
  - bf16 operands everywhere on the matmul paths (q/k/v/P/y/weights); fp32 PSUM
    accumulation. Halves HBM traffic and SBUF footprint; enables FWL weight loads.
  - t-chunk-major emission: A(qkv proj), B(attention) and C(out proj) are emitted
    interleaved (A0 A1 B0 A2 B1 C0 A3 B2 C1 B3 C2 C3) so the tensor engine never
    drains between phases and the ACT-paced B segments overlap A/C matmul work.
  - software-pipelined B inner loop: scores matmuls for s-block group g+1 are
    emitted before the AV matmuls of group g, hiding the exp (ACT) latency.
  - exp on [128,1024] blocks (two s-blocks per activation) to amortize the
    ~352-cycle ACT instruction overhead; causal masking via a DVE multiply with
    a precomputed 0/1 bf16 mask tile (diag blocks only; all-SBUF 2-byte
    operands hit the DVE 4x mode, ~4x faster than gpsimd affine_select);
    softmax denominator via an extra ones-column in the packed v operand
    (row 64 of the yT_aug accumulator).
  - bf16 partial outputs; host accumulates in fp32.
"""

import contextlib
import os
import sys

for _p in ("/opt/trn_rl_repo", "/root/.axon_site/_ro/trn_rl_repo"):
    if os.path.isdir(_p) and _p not in sys.path:
        sys.path.insert(0, _p)
        break

import numpy as np
import ml_dtypes

import concourse.bass as bass  # noqa: F401
import concourse.mybir as mybir
import concourse.tile as tile
from concourse import bacc
from concourse.bass_utils import run_bass_kernel_spmd

FP32 = mybir.dt.float32
FP32R = mybir.dt.float32r
BF16 = mybir.dt.bfloat16

B, T, C = 4, 2048, 768
NH, D = 12, 64
NCORES = 8
NKC = C // 128          # 6 contraction chunks
NTB = T // 128          # 16 s blocks
TCW = 512
NTC = T // TCW          # 4 t chunks
HPC = 6                 # heads per core
VW = HPC * 65           # 390: v block row width (6 heads x (64 + ones col))
WQKV = 3 * HPC * D      # 1152

_BUILT = None


def _build_nc(loops=1, phases="ABC"):
    nc = bacc.Bacc("TRN2", target_bir_lowering=False, debug=False, num_devices=NCORES)

    xT = nc.dram_tensor("xT", [C, T], BF16, kind="ExternalInput")
    wqkv = nc.dram_tensor("wqkv", [C, WQKV], BF16, kind="ExternalInput")
    bqk = nc.dram_tensor("bqk", [128, 6], FP32, kind="ExternalInput")
    qmask = nc.dram_tensor("qmask", [128, 4 * TCW], BF16, kind="ExternalInput")
    bvb = nc.dram_tensor("bvb", [128, 384], FP32, kind="ExternalInput")
    wproj = nc.dram_tensor("wproj", [384, C], BF16, kind="ExternalInput")
    part = nc.dram_tensor("part", [T, C], BF16, kind="ExternalOutput")

    with tile.TileContext(nc) as tc:
        def loop_cm(n):
            return tc.For_i(0, n, 1) if n > 1 else contextlib.nullcontext()

        with tc.sbuf_pool(name="pers", bufs=1) as pers, \
             tc.sbuf_pool(name="work", bufs=1) as work, \
             tc.psum_pool(name="ps", bufs=1) as ps:
            xt = pers.tile([128, NKC * T], BF16)
            wa = pers.tile([128, NKC * WQKV], BF16)
            qT = pers.tile([128, 3 * T], BF16)
            kT = pers.tile([128, 3 * T], BF16)
            vaug = pers.tile([128, NTB * VW], BF16)
            yT = pers.tile([128, 3 * T], BF16)
            wp = pers.tile([128, 3 * C], BF16)
            bqk_sb = pers.tile([128, 6], FP32)
            bvb_sb = pers.tile([128, 384], FP32)
            qmask_sb = pers.tile([128, 4 * TCW], BF16)
            nc.sync.dma_start(out=qmask_sb, in_=qmask[:, :])

            nc.sync.dma_start(
                out=wp.rearrange("p (k c) -> p k c", c=C),
                in_=wproj.rearrange("(k p) c -> p k c", p=128),
            )
            nc.sync.dma_start(out=bqk_sb, in_=bqk[:, :])
            nc.sync.dma_start(out=bvb_sb, in_=bvb[:, :])
            ones_cols = vaug.rearrange("p (g c) -> p g c", c=65)[:, :, 64:65]
            nc.vector.memset(ones_cols, 1.0)

            def a_qk_chain(tcw, pp, qk):
                acc = ps.tile([128, TCW], FP32, tag="mm", bufs=2, name="accqk")
                for kc in range(NKC):
                    co = kc * WQKV + qk * 384 + pp * 128
                    nc.tensor.matmul(
                        acc,
                        wa[:, co:co + 128],
                        xt[:, kc * T + tcw * TCW: kc * T + (tcw + 1) * TCW],
                        start=(kc == 0),
                        stop=(kc == NKC - 1),
                    )
                dst = qT if qk == 0 else kT
                nc.vector.tensor_scalar_add(
                    dst[:, pp * T + tcw * TCW: pp * T + (tcw + 1) * TCW],
                    acc,
                    bqk_sb[:, 3 * qk + pp: 3 * qk + pp + 1],
                )

            def a_v_chain(tb):
                accv = ps.tile([128, TCW], FP32, tag="mm", bufs=2, name="accv")
                for kc in range(NKC):
                    nc.tensor.matmul(
                        accv[:, 0:384],
                        xt[:, kc * T + tb * 128: kc * T + (tb + 1) * 128],
                        wa[:, kc * WQKV + 768: kc * WQKV + WQKV],
                        start=(kc == 0),
                        stop=(kc == NKC - 1),
                    )
                vdst = vaug[:, tb * VW:(tb + 1) * VW]
                nc.vector.tensor_tensor(
                    out=vdst.rearrange("p (h c) -> p h c", c=65)[:, :, 0:64],
                    in0=accv[:, 0:384].rearrange("p (h c) -> p h c", c=64),
                    in1=bvb_sb.rearrange("p (h c) -> p h c", c=64),
                    op=mybir.AluOpType.add,
                )

            def a_units(tcw):
                us = []
                for pp in range(3):
                    for qk in range(2):
                        us.append(lambda pp=pp, qk=qk: a_qk_chain(tcw, pp, qk))
                for tb in range(4 * tcw, 4 * tcw + 4):
                    us.append(lambda tb=tb: a_v_chain(tb))
                return us

            def c_tb(tb):
                osb = work.tile([128, C], BF16, tag="osb", bufs=2, name="osb")
                for ncw in range(2):
                    acc = ps.tile([128, TCW], FP32, tag="mm", bufs=2, name="accc")
                    for cc in range(3):
                        nc.tensor.matmul(
                            acc[:, 0:384],
                            yT[:, cc * T + tb * 128: cc * T + (tb + 1) * 128],
                            wp[:, cc * C + ncw * 384: cc * C + (ncw + 1) * 384],
                            start=(cc == 0),
                            stop=(cc == 2),
                        )
                    nc.vector.tensor_copy(
                        osb[:, ncw * 384:(ncw + 1) * 384], acc[:, 0:384])
                nc.sync.dma_start(out=part[tb * 128:(tb + 1) * 128, :], in_=osb)

            def c_units(tcw):
                return [lambda tb=tb: c_tb(tb)
                        for tb in range(4 * tcw, 4 * tcw + 4)]

            def emit_B(tcw, fillers=()):
                nsb = 4 * tcw + 4
                fillers = list(fillers)
                ngroups = 3 * (nsb // 2)
                popped = 0
                gidx = 0
                for pp in range(3):
                    yps = [ps.tile([65, TCW], FP32, tag="yt", bufs=2,
                                   name=f"yps{e}")
                           for e in range(2)]

                    def scores_group(g):
                        """4 score MMs + 2 exps (+ affine masks); returns P per e."""
                        out = []
                        for e in range(2):
                            pt = ps.tile([128, 2 * TCW], FP32, tag="pt", bufs=2)
                            for j in range(2):
                                sb = 2 * g + j
                                nc.tensor.matmul(
                                    pt[:, j * TCW:(j + 1) * TCW],
                                    kT[e * 64:(e + 1) * 64,
                                       pp * T + sb * 128: pp * T + (sb + 1) * 128],
                                    qT[e * 64:(e + 1) * 64,
                                       pp * T + tcw * TCW: pp * T + (tcw + 1) * TCW],
                                    start=True,
                                    stop=True,
                                )
                            P = work.tile([128, 2 * TCW], BF16, tag="p", bufs=4)
                            nc.scalar.activation(
                                P, pt, mybir.ActivationFunctionType.Exp, scale=0.125,
                            )
                            for j in range(2):
                                sb = 2 * g + j
                                if sb >= 4 * tcw:  # diagonal-band block
                                    jj = sb - 4 * tcw
                                    psl = P[:, j * TCW:(j + 1) * TCW]
                                    # causal mask as bf16 0/1 multiply on DVE
                                    # (all-SBUF 2-byte -> 4x mode, ~4x faster
                                    # than gpsimd affine_select)
                                    nc.vector.tensor_tensor(
                                        out=psl, in0=psl,
                                        in1=qmask_sb[:, jj * TCW:(jj + 1) * TCW],
                                        op=mybir.AluOpType.mult,
                                    )
                            out.append(P)
                        return out

                    def av_group(g, Ps):
                        for e in range(2):
                            h = 2 * pp + e
                            for j in range(2):
                                sb = 2 * g + j
                                nc.tensor.matmul(
                                    yps[e],
                                    vaug[:, sb * VW + h * 65: sb * VW + (h + 1) * 65],
                                    Ps[e][:, j * TCW:(j + 1) * TCW],
                                    start=(sb == 0),
                                    stop=(sb == nsb - 1),
                                )

                    prev = None
                    for g in range(nsb // 2):
                        cur = (g, scores_group(g))
                        if prev is not None:
                            av_group(*prev)
                        prev = cur
                        # spread filler PE work (A/C chains) between groups to
                        # keep the tensor engine busy while ACT/Pool convert
                        gidx += 1
                        want = (len(fillers) * gidx) // ngroups
                        while popped < want:
                            fillers[popped]()
                            popped += 1
                    av_group(*prev)

                    for e in range(2):
                        rt = work.tile([1, TCW], FP32R, tag="rt", bufs=2)
                        with nc.allow_low_precision(reason="fp32r softmax denom"):
                            nc.vector.reciprocal(rt, yps[e][64:65, :])
                        rbc = work.tile([64, TCW], FP32R, tag="rbc", bufs=2)
                        nc.gpsimd.partition_broadcast(rbc, rt)
                        with nc.allow_low_precision(reason="bf16 attn out"):
                            nc.vector.tensor_tensor(
                                out=yT[e * 64:(e + 1) * 64,
                                       pp * T + tcw * TCW: pp * T + (tcw + 1) * TCW],
                                in0=yps[e][0:64, :],
                                in1=rbc,
                                op=mybir.AluOpType.mult,
                            )
                while popped < len(fillers):
                    fillers[popped]()
                    popped += 1

            with loop_cm(loops):
                # input DMAs, weights first then x t-chunk-major
                for kc in range(NKC):
                    nc.sync.dma_start(
                        out=wa[:, kc * WQKV:(kc + 1) * WQKV],
                        in_=wqkv[kc * 128:(kc + 1) * 128, :],
                    )
                for tcw in range(NTC):
                    for kc in range(NKC):
                        nc.sync.dma_start(
                            out=xt[:, kc * T + tcw * TCW: kc * T + (tcw + 1) * TCW],
                            in_=xT[kc * 128:(kc + 1) * 128,
                                   tcw * TCW:(tcw + 1) * TCW],
                        )
                if "B" in phases:
                    for u in a_units(0):
                        u()
                    for tcw in range(NTC):
                        fillers = []
                        if tcw + 1 < NTC:
                            fillers += a_units(tcw + 1)
                        if "C" in phases and tcw >= 1:
                            fillers += c_units(tcw - 1)
                        emit_B(tcw, fillers)
                    if "C" in phases:
                        for u in c_units(NTC - 1):
                            u()
                else:  # A only (timing variant)
                    for tcw in range(NTC):
                        for u in a_units(tcw):
                            u()
                if "C" not in phases or "B" not in phases:
                    z = work.tile([128, 512], BF16, tag="osb0", bufs=1)
                    nc.vector.memset(z, 0.0)
                    nc.sync.dma_start(out=part[0:128, 0:512], in_=z)

    nc.compile()
    return nc


def _get_nc():
    global _BUILT
    if _BUILT is None:
        _BUILT = _build_nc()
    return _BUILT


def build_in_maps(x, W_attn, b_attn, W_proj):
    bf = ml_dtypes.bfloat16
    in_maps = []
    for c in range(NCORES):
        b = c // 2
        hs = (c % 2) * HPC
        q0, k0, v0 = hs * D, C + hs * D, 2 * C + hs * D
        w = HPC * D  # 384
        xT_b = np.ascontiguousarray(x[b].T).astype(bf)
        wqkv_c = np.ascontiguousarray(
            np.concatenate(
                [W_attn[:, q0:q0 + w], W_attn[:, k0:k0 + w], W_attn[:, v0:v0 + w]],
                axis=1,
            )
        ).astype(bf)
        bqk_c = np.stack(
            [b_attn[q0 + pp * 128: q0 + (pp + 1) * 128] for pp in range(3)]
            + [b_attn[k0 + pp * 128: k0 + (pp + 1) * 128] for pp in range(3)],
            axis=1,
        ).astype(np.float32)
        bvb_c = np.ascontiguousarray(
            np.broadcast_to(b_attn[v0:v0 + w][None, :], (128, w))
        ).astype(np.float32)
        wproj_c = np.ascontiguousarray(W_proj[hs * D: hs * D + w, :]).astype(bf)
        tt = np.arange(TCW)[None, :]
        ss = np.arange(128)[:, None]
        qmask_c = np.concatenate(
            [(tt >= jj * 128 + ss) for jj in range(4)], axis=1
        ).astype(bf)
        in_maps.append(
            {
                "xT": xT_b,
                "wqkv": wqkv_c,
                "bqk": bqk_c,
                "bvb": bvb_c,
                "wproj": wproj_c,
                "qmask": qmask_c,
            }
        )
    return in_maps


def kernel(x, W_attn, b_attn, W_proj, b_proj):
    x = np.asarray(x, dtype=np.float32)
    W_attn = np.asarray(W_attn, dtype=np.float32)
    b_attn = np.asarray(b_attn, dtype=np.float32)
    W_proj = np.asarray(W_proj, dtype=np.float32)
    b_proj = np.asarray(b_proj, dtype=np.float32)

    nc = _get_nc()
    in_maps = build_in_maps(x, W_attn, b_attn, W_proj)
    res = run_bass_kernel_spmd(nc, in_maps, core_ids=list(range(NCORES)))
    out = np.empty((B, T, C), dtype=np.float32)
    for b in range(B):
        out[b] = (
            res.results[2 * b]["part"].astype(np.float32)
            + res.results[2 * b + 1]["part"].astype(np.float32)
            + b_proj[None, :]
        )
    return out
